# revision 1
# baseline (speedup 1.0000x reference)
"""Distributed ADDSGNLM (guided non-local-means diffusion) on 8 NeuronCores.

Strategy (per sharding_hint): pure data parallel — the batch dim of y
(B=8) is sharded one image per core; h and s are tiny replicated params.
Each core runs the full per-image pipeline (Laplacian -> 4 rounds of
guided NLM over a 9x9 window with 7x7 patch distances); results are
gathered back to the full (8,1,1024,1024) output. Batch elements are
fully independent in the reference, so this sharding is exact (no halo
exchange needed).
"""

import numpy as np
import jax
import jax.numpy as jnp
from jax import lax
from functools import partial

EPS = 1e-8
PATCH_RAD = 3
WINDOW_RAD = 4

B, C, H, W = 8, 1, 1024, 1024
N_CORES = 8


def _box_mean(x, r):
    # separable (2r+1)x(2r+1) uniform patch average, reflect-padded. x: [B,C,H,W]
    k = 2 * r + 1
    xp = jnp.pad(x, ((0, 0), (0, 0), (r, r), (r, r)), mode='reflect')
    wh = jnp.ones((1, 1, k, 1), x.dtype) / k
    ww = jnp.ones((1, 1, 1, k), x.dtype) / k
    x = lax.conv_general_dilated(xp, wh, (1, 1), 'VALID')
    x = lax.conv_general_dilated(x, ww, (1, 1), 'VALID')
    return x


def _laplacian(x):
    kern = jnp.array([[0.0, 1.0, 0.0], [1.0, -4.0, 1.0], [0.0, 1.0, 0.0]],
                     x.dtype).reshape(1, 1, 3, 3)
    xp = jnp.pad(x, ((0, 0), (0, 0), (1, 1), (1, 1)), mode='reflect')
    return lax.conv_general_dilated(xp, kern, (1, 1), 'VALID')


def _dsgnlm(noisy, guide, patch_rad, window_rad, sigma):
    b, c, h_, w_ = noisy.shape
    wr = window_rad
    npad = jnp.pad(noisy, ((0, 0), (0, 0), (wr, wr), (wr, wr)), mode='reflect')
    gpad = jnp.pad(guide, ((0, 0), (0, 0), (wr, wr), (wr, wr)), mode='reflect')
    sigma2 = sigma * sigma + EPS
    offs = jnp.array([(dy, dx) for dy in range(2 * wr + 1)
                      for dx in range(2 * wr + 1)], dtype=jnp.int32)

    def body(carry, off):
        num, den = carry
        dy, dx = off[0], off[1]
        g_sh = lax.dynamic_slice(gpad, (0, 0, dy, dx), (b, c, h_, w_))
        n_sh = lax.dynamic_slice(npad, (0, 0, dy, dx), (b, c, h_, w_))
        d = _box_mean((guide - g_sh) ** 2, patch_rad)
        w = jnp.exp(-d / sigma2)
        return (num + w * n_sh, den + w), None

    init = (jnp.zeros_like(noisy), jnp.zeros_like(noisy))
    (num, den), _ = lax.scan(body, init, offs)
    return num / den


def _per_image(y, h, s):
    # y: [1,1,H,W] — one batch element on one core
    lap = _laplacian(y)
    x = _dsgnlm(y, y, PATCH_RAD, WINDOW_RAD, h[0] * lap)
    for i in range(3):
        step = jnp.clip(s[i], 0.6, 1.0)
        x = (1.0 - step) * x + step * y
        lap = _laplacian(x)
        x = _dsgnlm(x, y, PATCH_RAD, WINDOW_RAD, h[i + 1] * lap)
    return x


_PMAPPED = None


def _get_pmapped(n_dev):
    global _PMAPPED
    if _PMAPPED is None:
        _PMAPPED = jax.pmap(_per_image, axis_name='b',
                            devices=jax.devices()[:n_dev])
    return _PMAPPED


def kernel(y: np.ndarray, h: np.ndarray, s: np.ndarray) -> np.ndarray:
    y = np.asarray(y, dtype=np.float32)
    h = np.asarray(h, dtype=np.float32)
    s = np.asarray(s, dtype=np.float32)
    n_dev = min(len(jax.devices()), y.shape[0])
    if y.shape[0] % n_dev != 0:
        n_dev = 1
    # shard batch across cores: [B,1,H,W] -> [n_dev, B/n_dev, 1, H, W]
    per = y.shape[0] // n_dev
    y_sh = y.reshape(n_dev, per, *y.shape[1:])
    h_rep = np.broadcast_to(h, (n_dev,) + h.shape)
    s_rep = np.broadcast_to(s, (n_dev,) + s.shape)
    fn = _get_pmapped(n_dev)
    out = fn(y_sh, h_rep, s_rep)
    out = np.asarray(out)
    return out.reshape(y.shape).astype(np.float32)
